# revision 25
# baseline (speedup 1.0000x reference)
"""Trainium2 Bass kernel for the MHSA bottleneck block.

Contract: kernel(**inputs) takes the FULL unsharded inputs (as produced by
setup_inputs()) and returns the FULL [64, 2048, 14, 14] float32 output.
Internally shards data-parallel over batch: 8 images per NeuronCore, 8 cores.

v5: fp8 DoubleRow conv1 + q/k projection (on top of v3's fp8
value/attention/conv3), identity-matmul residual with host-prefolded
xr = x + t3 (bare-relu epilogue), x8-prescaled fp8 weights (undone via
activation scales), and partition-major DRAM layouts so DMA descriptors move
3-12KB contiguous runs per partition instead of 784B (the v4 startup was
DMA-packet-bound at ~40GB/s).
"""
import sys

sys.path.insert(0, '/opt/trn_rl_repo')

import numpy as np
import ml_dtypes

# Problem constants (hardcoded per the harness contract).
B, CIN, P, H, W = 64, 2048, 512, 14, 14
EPS = 1e-5
N = H * W            # 196 pixels
NCORES = 8
BPC = B // NCORES    # 8 images per core
NPAIR = BPC // 2     # 4 image pairs per core
KC1 = CIN // 128     # 16 input-channel chunks (bf16 view) for residual/y
KC2 = CIN // 256     # 8 DoubleRow input-channel chunks for conv1
PC = P // 128        # 4 chunks of the 512-dim
N2 = 2 * N           # 392 = free dim for image-pair matmuls
WS = 8.0             # host-side fp8 weight pre-scale (undone on-chip)

# m chunking of the 196-pixel dim: 128 + 68
NCHUNKS = [(0, 128), (128, 68)]

_CACHE = {}


def _build():
    import concourse.bass as bass  # noqa: F401
    import concourse.mybir as mybir
    import concourse.tile as tile
    from concourse import bacc

    f32 = mybir.dt.float32
    bf16 = mybir.dt.bfloat16
    f8 = mybir.dt.float8e4

    DR = mybir.MatmulPerfMode.DoubleRow

    nc = bacc.Bacc(None, target_bir_lowering=False, debug=False)

    # fp8 x, partition-major: [pair, p, kc2*784 + sub*392 + j*196 + n]
    x8_d = nc.declare_dram_parameter("x8", [NPAIR, 128, KC2 * 784], f8,
                                     isOutput=False)
    # bf16 x with t3 folded in: [pair, p, kc1*392 + j*196 + n]
    xr_d = nc.declare_dram_parameter("xr", [NPAIR, 128, KC1 * N2], bf16,
                                     isOutput=False)
    # conv1 weights fp8 x8-scaled: [ocb, p, kc2*256 + sub*128 + ocm]
    w18_d = nc.declare_dram_parameter("w18", [PC, 128, KC2 * 256], f8,
                                      isOutput=False)
    # qk weights fp8 x8-scaled: [p, ocb, kc2*256 + sub*128 + ocm]
    wqk8_d = nc.declare_dram_parameter("wqk8", [128, 8, 512], f8,
                                       isOutput=False)
    # value weights fp8 x8-scaled: [p, pc, P]
    wvt_d = nc.declare_dram_parameter("wvt", [128, PC, P], f8, isOutput=False)
    # conv3 weights fp8 (unscaled): [p, pc, CIN]
    w3t_d = nc.declare_dram_parameter("w3t", [128, PC, CIN], f8,
                                      isOutput=False)
    pos_d = nc.declare_dram_parameter("pos", [128, PC, N], bf16,
                                      isOutput=False)
    # packed per-channel bias/scale vectors: t1 | s2/8
    tb_d = nc.declare_dram_parameter("tb", [128, 2 * PC], f32, isOutput=False)
    # 8*t2/s2 as an fp8 row, injected via the attention pad row
    t2v_d = nc.declare_dram_parameter("t2v", [1, P], f8, isOutput=False)
    # output, partition-major: [pair, p, kc1, j*196 + n]
    y_d = nc.declare_dram_parameter("y", [NPAIR, 128, KC1, N2], bf16,
                                    isOutput=True)

    with tile.TileContext(nc) as tc:
        with (
            tc.tile_pool(name="const", bufs=1) as const,
            tc.tile_pool(name="x8p", bufs=3) as x8p,
            tc.tile_pool(name="xrp", bufs=2) as xrp,
            tc.tile_pool(name="h18p", bufs=2) as h18p,
            tc.tile_pool(name="qkp", bufs=2) as qkp,
            tc.tile_pool(name="h2p", bufs=2) as h2p,
            tc.tile_pool(name="attp", bufs=2) as attp,
            tc.tile_pool(name="outp", bufs=4) as outp,
            tc.tile_pool(name="ps_mm", bufs=5, space="PSUM") as ps_mm,
            tc.tile_pool(name="ps_sm", bufs=3, space="PSUM") as ps_sm,
        ):
            Exp = mybir.ActivationFunctionType.Exp
            Relu = mybir.ActivationFunctionType.Relu
            Copy = mybir.ActivationFunctionType.Copy

            S = [dict() for _ in range(NPAIR)]

            # ---------------- DMA emitters ----------------
            # Each dma_start costs ~700ns of serial issue time on its queue,
            # so startup batches transfers into few calls and spreads issues
            # across queues (tb/t2v on vector, pair-1 x8 on scalar).
            def emit_x8_dma(p, eng=None):
                t = x8p.tile([128, KC2, 2, N2], f8, name=f"x8_{p}", tag="x8")
                S[p]['x8'] = t
                (eng or nc.sync).dma_start(out=t, in_=x8_d[p, :, :])

            def emit_xr_dma(p, eng=None):
                t = xrp.tile([128, KC1, N2], bf16, name=f"xr_{p}", tag="xr")
                S[p]['xr'] = t
                (eng or nc.gpsimd).dma_start(out=t, in_=xr_d[p, :, :])

            # pair-0 x8 in two halves (kc0-3 | kc4-7): 3.1KB runs per
            # partition keep the DMA out of its per-packet-bound regime
            x8c0 = [const.tile([128, 4, 2, N2], f8, name="x8c0a"),
                    const.tile([128, 4, 2, N2], f8, name="x8c0b")]
            w18a = const.tile([128, KC2 * 256], f8, name="w18a")
            w18b = const.tile([128, 3, KC2 * 256], f8, name="w18b")

            def w18sl(oc, kc):
                t = w18a if oc == 0 else w18b[:, oc - 1, :]
                return t[:, kc * 256:(kc + 1) * 256].rearrange(
                    "p (s m) -> p s m", s=2)

            def x8sl(p, kc):
                if p > 0:
                    return S[p]['x8'][:, kc, :, :]
                return x8c0[kc // 4][:, kc % 4, :, :]

            nc.sync.dma_start(out=w18a, in_=w18_d[0, :, :])
            nc.sync.dma_start(out=x8c0[0], in_=x8_d[0, :, 0:4 * 784])
            tb = const.tile([128, 2 * PC], f32)
            nc.scalar.dma_start(out=tb, in_=tb_d[:, :])
            t2v = const.tile([1, P], f8)
            nc.scalar.dma_start(out=t2v, in_=t2v_d[:, :])
            t1 = tb[:, 0:PC]
            s2 = tb[:, PC:2 * PC]
            nc.sync.dma_start(out=x8c0[1], in_=x8_d[0, :, 4 * 784:8 * 784])
            nc.sync.dma_start(
                out=w18b, in_=w18_d[1:4, :, :].rearrange("o p s -> p o s"))
            wqk8 = const.tile([128, 8, 512], f8)
            nc.sync.dma_start(out=wqk8, in_=wqk8_d[:, :, :])
            # pair-1 x8 stays on sync BEHIND the pair-0 critical transfers;
            # a second queue would steal DMA bandwidth from them
            emit_x8_dma(1)

            wvt = const.tile([128, PC, P], f8)
            pos = const.tile([128, PC, N], bf16)
            w3t = const.tile([128, PC, CIN], f8)
            ones_sb = const.tile([128, 128], bf16)
            nc.gpsimd.memset(ones_sb, 1.0)
            from concourse.masks import make_identity
            identb = const.tile([128, 128], bf16)
            make_identity(nc, identb)

            def emit_late_weights():
                nc.sync.dma_start(out=wvt, in_=wvt_d[:, :, :])
                nc.sync.dma_start(out=pos, in_=pos_d[:, :, :])
                nc.sync.dma_start(out=w3t, in_=w3t_d[:, :, :])

            # ---------------- block emitters ----------------
            def conv1_block(p, oc):
                Sp = S[p]
                if oc == 0:
                    # cols = j*256 + n (per-image padded to 256 so the vT
                    # stationary slices meet the DR 16B step alignment)
                    Sp['h18'] = h18p.tile([128, 2, 2, 512], f8,
                                          name=f"h18_{p}", tag="h18")
                cps = ps_mm.tile([128, 512], f32, name="cps", tag="mm")
                for kc in range(KC2):
                    nc.tensor.matmul(
                        cps[:, :N2],
                        w18sl(oc, kc),
                        x8sl(p, kc),
                        start=(kc == 0), stop=(kc == KC2 - 1),
                        perf_mode=DR,
                    )
                # h18 = relu(cps/8 + t1), straight to fp8
                nc.scalar.activation(
                    Sp['h18'][:, oc // 2, oc % 2, :].rearrange(
                        "p (j w) -> p j w", j=2)[:, :, :N],
                    cps[:, :N2].rearrange("p (j n) -> p j n", j=2),
                    Relu, bias=t1[:, oc:oc + 1], scale=1.0 / WS)

            def qk_block(p, oc):
                Sp = S[p]
                if oc == 0:
                    Sp['q'] = qkp.tile([128, PC, 2, N], bf16,
                                       name=f"q_{p}", tag="q")
                    Sp['k'] = qkp.tile([128, PC, 2, N], bf16,
                                       name=f"k_{p}", tag="k")
                qps = ps_mm.tile([128, 512], f32, name="qps", tag="mm")
                for kc in range(2):
                    nc.tensor.matmul(
                        qps[:, :],
                        wqk8[:, oc, kc * 256:(kc + 1) * 256].rearrange(
                            "p (s m) -> p s m", s=2),
                        Sp['h18'][:, kc, :, :],
                        start=(kc == 0), stop=(kc == 1),
                        perf_mode=DR,
                    )
                dst = Sp['q'] if oc < PC else Sp['k']
                c4 = oc % PC
                qv = qps[:, :].rearrange("p (j w) -> p j w", j=2)[:, :, :N]
                if oc % 2 == 0:
                    nc.scalar.activation(dst[:, c4, :, :], qv,
                                         Copy, scale=1.0 / WS)
                else:
                    nc.vector.tensor_scalar_mul(dst[:, c4, :, :], qv,
                                                1.0 / WS)

            def vT_block(p, j):
                Sp = S[p]
                # vT holds 8*v (wvt pre-scaled); undone in the h2 activation
                vT = attp.tile([128, 2, P], f8, name=f"vT_{p}_{j}", tag="vT")
                Sp[f'vT{j}'] = vT
                # rows 68.. of the second m-chunk stay zero (m=196..255 pad);
                # vT pad rows are garbage-filled, so zero them too (fp8 NaN
                # times attn 0 would poison the aout contraction).
                # Pad row 96 (m=224) carries the h2 bias: attn=1, vT=8*t2/s2,
                # so the aout matmul itself adds t2 and the h2 drain is a
                # bias-free relu*scale that either engine can run.
                # rows 64:68 are re-written by the vps copy below
                nc.gpsimd.memset(vT[64:128, 1, :], 0.0)
                nc.vector.tensor_copy(vT[96:97, 1, :], t2v[0:1, :])
                for mi, (m0, msz) in enumerate(NCHUNKS):
                    vps = ps_mm.tile([128, 512], f32, name="vps", tag="mm")
                    for i in range(2):
                        nc.tensor.matmul(
                            vps[:msz, :],
                            Sp['h18'][:, i, :,
                                      j * 256 + m0:j * 256 + m0 + msz],
                            wvt[:, 2 * i:2 * i + 2, :],
                            start=(i == 0), stop=(i == 1),
                            perf_mode=DR,
                        )
                    nc.vector.tensor_copy(vT[:msz, mi, :], vps[:msz, :])

            def sT_block(p, j, mi):
                Sp = S[p]
                if j == 0 and mi == 0:
                    # per-pair exp/attn tiles: cols = (j, n) contiguous
                    Sp['expT'] = attp.tile([128, 2, 2, N], bf16,
                                           name=f"eT_{p}", tag="expT")
                    Sp['attnT'] = attp.tile([128, 2, 2, N], f8,
                                            name=f"aT_{p}", tag="attnT")
                    nc.gpsimd.memset(Sp['attnT'][64:128, 1, :, :], 0.0)
                    nc.gpsimd.memset(Sp['attnT'][96:97, 1, :, :], 1.0)
                m0, msz = NCHUNKS[mi]
                q, k = Sp['q'], Sp['k']
                lps = ps_sm.tile([128, 256], f32, name="lps", tag="small")
                # scores transposed: sT[m, n] = sum_c k[c,m] q[c,n]
                #                             + sum_c q[c,m] pos[c,n]
                for pc in range(PC):
                    nc.tensor.matmul(
                        lps[:msz, :N],
                        k[:, pc, j, m0:m0 + msz],
                        q[:, pc, j, :],
                        start=(pc == 0), stop=False,
                    )
                for pc in range(PC):
                    nc.tensor.matmul(
                        lps[:msz, :N],
                        q[:, pc, j, m0:m0 + msz],
                        pos[:, pc, :],
                        start=False, stop=(pc == PC - 1),
                    )
                # exp (no max subtraction: logits O(40) max, finite in fp32,
                # and bf16 holds e^40 fine)
                nc.scalar.activation(Sp['expT'][:msz, mi, j, :],
                                     lps[:msz, :N], Exp)

            def softsum_block(p):
                Sp = S[p]
                expT = Sp['expT']
                spsum = ps_sm.tile([1, 512], f32, name="spsum", tag="small")
                for mi, (m0, msz) in enumerate(NCHUNKS):
                    nc.tensor.matmul(
                        spsum[:1, :N2],
                        ones_sb[:msz, 0:1],
                        expT[:msz, mi, :, :],
                        start=(mi == 0), stop=(mi == 1),
                    )
                Sp['spsum'] = spsum

            def softnorm_block(p):
                Sp = S[p]
                rinv32 = attp.tile([1, N2], f32, name=f"rinv32_{p}",
                                   tag="rinv32")
                nc.vector.reciprocal_approx_fast(rinv32[:1, :],
                                                 Sp['spsum'][:1, :N2])
                rinv = attp.tile([1, N2], bf16, name=f"rinv_{p}", tag="rinv")
                nc.vector.tensor_copy(rinv[:1, :], rinv32[:1, :])
                rps = ps_sm.tile([128, 512], f32, name="rps", tag="small")
                nc.tensor.matmul(rps[:, :N2], ones_sb[0:1, :], rinv[:1, :],
                                 start=True, stop=True)
                expT, attnT = Sp['expT'], Sp['attnT']
                for mi, (m0, msz) in enumerate(NCHUNKS):
                    nc.vector.tensor_mul(attnT[:msz, mi, :, :],
                                         expT[:msz, mi, :, :],
                                         rps[:msz, :N2])

            def aout_block(p, j):
                Sp = S[p]
                if j == 0:
                    Sp['h2'] = h2p.tile([128, 2, 2, N2], f8,
                                        name=f"h2_{p}", tag="h2")
                vT, attnT = Sp[f'vT{j}'], Sp['attnT']
                for c4 in range(PC):
                    aps = ps_sm.tile([128, 256], f32, name="aps", tag="small")
                    nc.tensor.matmul(
                        aps[:, :N],
                        vT[:, :, c4 * 128:(c4 + 1) * 128],
                        attnT[:, :, j, :],
                        start=True, stop=True,
                        perf_mode=DR,
                    )
                    # h2 = relu((s2/8)*(aps + 8*t2/s2)) = relu(s2/8*aps + t2)
                    # (t2 comes in through the pad row; s2 pre-divided, >0)
                    dst = Sp['h2'][:, c4 // 2, c4 % 2, j * N:(j + 1) * N]
                    if c4 % 2 == 0:
                        nc.scalar.activation(dst, aps[:, :N], Relu,
                                             scale=s2[:, c4:c4 + 1])
                    else:
                        nc.vector.tensor_scalar(
                            dst, aps[:, :N], 0.0, s2[:, c4:c4 + 1],
                            op0=mybir.AluOpType.max,
                            op1=mybir.AluOpType.mult)

            def conv3_block(p, k4, final=False):
                Sp = S[p]
                y_sb = outp.tile([128, 4, N2], bf16, name="y_sb", tag="y_sb")
                # 2-oc groups with the two bf16 identity (residual) matmuls
                # back-to-back: half the PE DR<->bf16 mode transitions
                for h in range(2):
                    oc0 = 4 * k4 + 2 * h
                    opsl = [ps_mm.tile([128, 512], f32, name="ops", tag="mm")
                            for _ in range(2)]
                    for d in range(2):
                        for ch in range(2):
                            nc.tensor.matmul(
                                opsl[d][:, :N2],
                                w3t[:, 2 * ch:2 * ch + 2,
                                    (oc0 + d) * 128:(oc0 + d + 1) * 128],
                                Sp['h2'][:, ch, :, :],
                                start=(ch == 0), stop=False,
                                perf_mode=DR, skip_group_check=True,
                            )
                    for d in range(2):
                        # residual + t3 folded in via identity matmul on xr
                        nc.tensor.matmul(opsl[d][:, :N2], identb[:, :],
                                         Sp['xr'][:, oc0 + d, :],
                                         start=False, stop=True,
                                         skip_group_check=True)
                    for d in range(2):
                        i4 = 2 * h + d
                        if d == 0:
                            nc.scalar.activation(y_sb[:, i4, :],
                                                 opsl[d][:, :N2], Relu)
                        else:
                            nc.vector.tensor_scalar_max(y_sb[:, i4, :],
                                                        opsl[d][:, :N2], 0.0)
                if final:
                    # drain the last stores per-2-oc on two queues so they
                    # overlap the remaining relus
                    for h in range(2):
                        eng = nc.sync if h == 0 else nc.gpsimd
                        nc_eng = eng
                        nc_eng.dma_start(
                            out=y_d[p, :, 4 * k4 + 2 * h:4 * k4 + 2 * h + 2,
                                    :],
                            in_=y_sb[:, 2 * h:2 * h + 2, :])
                else:
                    nc.gpsimd.dma_start(
                        out=y_d[p, :, 4 * k4:4 * k4 + 4, :],
                        in_=y_sb[:, :, :])

            # ---------------- pipeline driver ----------------
            def A_blocks(p):
                return ([lambda p=p, oc=oc: conv1_block(p, oc)
                         for oc in range(PC)] +
                        [lambda p=p, oc=oc: qk_block(p, oc)
                         for oc in range(2 * PC)])

            def B_blocks(p, final=False):
                # scores first, then the pair-wide softsum; the vT blocks sit
                # between softsum and softnorm to hide the reciprocal chain
                out = []
                for step in (lambda p, j: sT_block(p, j, 0),
                             lambda p, j: sT_block(p, j, 1)):
                    for j in range(2):
                        out.append(lambda p=p, j=j, s=step: s(p, j))
                out.append(lambda p=p: softsum_block(p))
                for j in range(2):
                    out.append(lambda p=p, j=j: vT_block(p, j))
                out.append(lambda p=p: softnorm_block(p))
                for j in range(2):
                    out.append(lambda p=p, j=j: aout_block(p, j))
                out += [lambda p=p, k=k: conv3_block(p, k, final)
                        for k in range(4)]
                return out

            def interleave(Bl, Al):
                nB, nA = len(Bl), len(Al)
                ai = 0
                for bi, b in enumerate(Bl):
                    b()
                    target = ((bi + 1) * nA) // nB
                    while ai < target:
                        Al[ai]()
                        ai += 1
                while ai < nA:
                    Al[ai]()
                    ai += 1

            prevB = None
            for p in range(NPAIR):
                A = A_blocks(p)
                if prevB is None:
                    for idx, a in enumerate(A):
                        a()
                        if idx == 1:
                            emit_late_weights()
                            # xr(0) on sync too: gpsimd would start it
                            # immediately and compete with pair-0 transfers
                            emit_xr_dma(0, eng=nc.sync)
                else:
                    # prefetch ahead of this iteration's y stores
                    if p + 1 < NPAIR:
                        emit_x8_dma(p + 1)
                    emit_xr_dma(p)
                    interleave(prevB, A)
                prevB = B_blocks(p, final=(p == NPAIR - 1))
            for b in prevB:
                b()

    nc.compile()
    return nc


def _prep_inputs(x, w1, g1, b1, m1, v1, wqkv, rel_h, rel_w,
                 g2, b2, m2, v2, w3, g3, b3, m3, v3):
    f = np.float32
    bf = ml_dtypes.bfloat16
    f8 = ml_dtypes.float8_e4m3
    s1 = (g1 / np.sqrt(v1 + EPS)).astype(f)
    t1 = (b1 - m1 * s1).astype(f)
    s2 = (g2 / np.sqrt(v2 + EPS)).astype(f)
    t2 = (b2 - m2 * s2).astype(f)
    s3 = (g3 / np.sqrt(v3 + EPS)).astype(f)
    t3 = (b3 - m3 * s3).astype(f)

    # conv1 weights: fold s1, pre-scale x8, DR stationary layout
    # [ocb, p, kc2*256 + sub*128 + ocm]; cin = kc2*256 + sub*128 + p
    w1p = (w1 * s1[:, None] * WS).astype(f)               # [512, 2048]
    w18 = w1p.reshape(PC, 128, KC2, 2, 128).transpose(0, 4, 2, 3, 1)
    w18 = np.ascontiguousarray(w18.reshape(PC, 128, KC2 * 256)).astype(f8)

    # qk weights: [p, ocb(8), kc2*256 + sub*128 + ocm]
    wqk = (wqkv[:2 * P] * WS).astype(f)                   # [1024, 512]
    wqk8 = wqk.reshape(8, 128, 2, 2, 128).transpose(4, 0, 2, 3, 1)
    wqk8 = np.ascontiguousarray(wqk8.reshape(128, 8, 512)).astype(f8)

    wv = (wqkv[2 * P:] * WS).astype(f)                    # [512, 512]
    wvt = np.ascontiguousarray(
        wv.T.reshape(PC, 128, P).transpose(1, 0, 2)).astype(f8)
    w3p = (w3 * s3[:, None]).astype(f)                    # [2048, 512]
    w3t = np.ascontiguousarray(
        w3p.T.reshape(PC, 128, CIN).transpose(1, 0, 2)).astype(f8)
    pos = (rel_h + rel_w).reshape(P, N).astype(f)
    pos = np.ascontiguousarray(
        pos.reshape(PC, 128, N).transpose(1, 0, 2)).astype(bf)

    tb = np.concatenate([t1.reshape(PC, 128).T,
                         (s2 / WS).reshape(PC, 128).T], axis=1)
    tb = np.ascontiguousarray(tb, f)
    t2v = np.ascontiguousarray((WS * t2 / s2).reshape(1, P)).astype(f8)

    shared = dict(w18=w18, wqk8=wqk8, wvt=wvt, w3t=w3t, pos=pos, tb=tb,
                  t2v=t2v)

    xf = np.asarray(x, f)
    in_maps = []
    for c in range(NCORES):
        xc = xf[c * BPC:(c + 1) * BPC].reshape(BPC, CIN, N)
        # fp8 copy, partition-major DR layout:
        # [pair, p, kc2*784 + sub*392 + j*196 + n]
        x8 = xc.reshape(NPAIR, 2, KC2, 2, 128, N).transpose(0, 4, 2, 3, 1, 5)
        x8 = np.ascontiguousarray(
            x8.reshape(NPAIR, 128, KC2 * 784)).astype(f8)
        # bf16 residual copy with t3: [pair, p, kc1*392 + j*196 + n]
        xr = xc + t3[None, :, None]
        xr = xr.reshape(NPAIR, 2, KC1, 128, N).transpose(0, 3, 2, 1, 4)
        xr = np.ascontiguousarray(
            xr.reshape(NPAIR, 128, KC1 * N2)).astype(bf)
        in_maps.append(dict(shared, x8=x8, xr=xr))
    return in_maps


def _run(in_maps, trace=False, tmpdir=None):
    from concourse.bass_utils import run_bass_kernel_spmd
    if "nc" not in _CACHE:
        _CACHE["nc"] = _build()
    nc = _CACHE["nc"]
    return run_bass_kernel_spmd(nc, in_maps, core_ids=list(range(NCORES)),
                                trace=trace, tmpdir=tmpdir)


def _post(res):
    out = np.empty((B, CIN, H, W), np.float32)
    for c in range(NCORES):
        # y: [pair, p, kc1, j*196 + n]
        yc = res.results[c]["y"].astype(np.float32).reshape(
            NPAIR, 128, KC1, 2, N)
        out[c * BPC:(c + 1) * BPC] = yc.transpose(0, 3, 2, 1, 4).reshape(
            BPC, CIN, H, W)
    return out


def kernel(**inputs):
    in_maps = _prep_inputs(**inputs)
    res = _run(in_maps)
    return _post(res)


# revision 27
# speedup vs baseline: 1.0259x; 1.0259x over previous
"""Trainium2 Bass kernel for the MHSA bottleneck block.

Contract: kernel(**inputs) takes the FULL unsharded inputs (as produced by
setup_inputs()) and returns the FULL [64, 2048, 14, 14] float32 output.
Internally shards data-parallel over batch: 8 images per NeuronCore, 8 cores.

v5: fp8 DoubleRow conv1 + q/k projection (on top of v3's fp8
value/attention/conv3), identity-matmul residual with host-prefolded
xr = x + t3 (bare-relu epilogue), x8-prescaled fp8 weights (undone via
activation scales), and partition-major DRAM layouts so DMA descriptors move
3-12KB contiguous runs per partition instead of 784B (the v4 startup was
DMA-packet-bound at ~40GB/s).
"""
import sys

sys.path.insert(0, '/opt/trn_rl_repo')

import numpy as np
import ml_dtypes

# Problem constants (hardcoded per the harness contract).
B, CIN, P, H, W = 64, 2048, 512, 14, 14
EPS = 1e-5
N = H * W            # 196 pixels
NCORES = 8
BPC = B // NCORES    # 8 images per core
NPAIR = BPC // 2     # 4 image pairs per core
KC1 = CIN // 128     # 16 input-channel chunks (bf16 view) for residual/y
KC2 = CIN // 256     # 8 DoubleRow input-channel chunks for conv1
PC = P // 128        # 4 chunks of the 512-dim
N2 = 2 * N           # 392 = free dim for image-pair matmuls
WS = 8.0             # host-side fp8 weight pre-scale (undone on-chip)

# m chunking of the 196-pixel dim: 128 + 68
NCHUNKS = [(0, 128), (128, 68)]

_CACHE = {}


def _build():
    import concourse.bass as bass  # noqa: F401
    import concourse.mybir as mybir
    import concourse.tile as tile
    from concourse import bacc

    f32 = mybir.dt.float32
    bf16 = mybir.dt.bfloat16
    f8 = mybir.dt.float8e4

    DR = mybir.MatmulPerfMode.DoubleRow

    nc = bacc.Bacc(None, target_bir_lowering=False, debug=False)

    # fp8 x, partition-major: [pair, p, kc2*784 + sub*392 + j*196 + n]
    x8_d = nc.declare_dram_parameter("x8", [NPAIR, 128, KC2 * 784], f8,
                                     isOutput=False)
    # bf16 x with t3 folded in: [pair, p, kc1*392 + j*196 + n]
    xr_d = nc.declare_dram_parameter("xr", [NPAIR, 128, KC1 * N2], bf16,
                                     isOutput=False)
    # conv1 weights fp8 x8-scaled: [ocb, p, kc2*256 + sub*128 + ocm]
    w18_d = nc.declare_dram_parameter("w18", [PC, 128, KC2 * 256], f8,
                                      isOutput=False)
    # qk weights fp8 x8-scaled: [p, ocb, kc2*256 + sub*128 + ocm]
    wqk8_d = nc.declare_dram_parameter("wqk8", [128, 8, 512], f8,
                                       isOutput=False)
    # value weights fp8 x8-scaled: [p, pc, P]
    wvt_d = nc.declare_dram_parameter("wvt", [128, PC, P], f8, isOutput=False)
    # conv3 weights fp8 (unscaled): [p, pc, CIN]
    w3t_d = nc.declare_dram_parameter("w3t", [128, PC, CIN], f8,
                                      isOutput=False)
    pos_d = nc.declare_dram_parameter("pos", [128, PC, N], bf16,
                                      isOutput=False)
    # packed per-channel bias/scale vectors: t1 | s2/8
    tb_d = nc.declare_dram_parameter("tb", [128, 2 * PC], f32, isOutput=False)
    # 8*t2/s2 as an fp8 row, injected via the attention pad row
    t2v_d = nc.declare_dram_parameter("t2v", [1, P], f8, isOutput=False)
    # output, partition-major: [pair, p, kc1, j*196 + n]
    y_d = nc.declare_dram_parameter("y", [NPAIR, 128, KC1, N2], bf16,
                                    isOutput=True)

    with tile.TileContext(nc) as tc:
        with (
            tc.tile_pool(name="const", bufs=1) as const,
            tc.tile_pool(name="x8p", bufs=3) as x8p,
            tc.tile_pool(name="xrp", bufs=2) as xrp,
            tc.tile_pool(name="h18p", bufs=2) as h18p,
            tc.tile_pool(name="qkp", bufs=2) as qkp,
            tc.tile_pool(name="h2p", bufs=2) as h2p,
            tc.tile_pool(name="attp", bufs=2) as attp,
            tc.tile_pool(name="outp", bufs=4) as outp,
            tc.tile_pool(name="ps_mm", bufs=5, space="PSUM") as ps_mm,
            tc.tile_pool(name="ps_sm", bufs=3, space="PSUM") as ps_sm,
        ):
            Exp = mybir.ActivationFunctionType.Exp
            Relu = mybir.ActivationFunctionType.Relu
            Copy = mybir.ActivationFunctionType.Copy

            S = [dict() for _ in range(NPAIR)]

            # ---------------- DMA emitters ----------------
            # Each dma_start costs ~700ns of serial issue time on its queue,
            # so startup batches transfers into few calls and spreads issues
            # across queues (tb/t2v on vector, pair-1 x8 on scalar).
            def emit_x8_dma(p, eng=None):
                t = x8p.tile([128, KC2, 2, N2], f8, name=f"x8_{p}", tag="x8")
                S[p]['x8'] = t
                (eng or nc.sync).dma_start(out=t, in_=x8_d[p, :, :])

            def emit_xr_dma(p, eng=None):
                t = xrp.tile([128, KC1, N2], bf16, name=f"xr_{p}", tag="xr")
                S[p]['xr'] = t
                (eng or nc.gpsimd).dma_start(out=t, in_=xr_d[p, :, :])

            # pair-0 x8 in two halves (kc0-3 | kc4-7): 3.1KB runs per
            # partition keep the DMA out of its per-packet-bound regime
            x8c0 = [const.tile([128, 4, 2, N2], f8, name="x8c0a"),
                    const.tile([128, 4, 2, N2], f8, name="x8c0b")]
            w18a = const.tile([128, KC2 * 256], f8, name="w18a")
            w18b = const.tile([128, 3, KC2 * 256], f8, name="w18b")

            def w18sl(oc, kc):
                t = w18a if oc == 0 else w18b[:, oc - 1, :]
                return t[:, kc * 256:(kc + 1) * 256].rearrange(
                    "p (s m) -> p s m", s=2)

            def x8sl(p, kc):
                if p > 0:
                    return S[p]['x8'][:, kc, :, :]
                return x8c0[kc // 4][:, kc % 4, :, :]

            nc.sync.dma_start(out=w18a, in_=w18_d[0, :, :])
            nc.sync.dma_start(out=x8c0[0], in_=x8_d[0, :, 0:4 * 784])
            tb = const.tile([128, 2 * PC], f32)
            nc.scalar.dma_start(out=tb, in_=tb_d[:, :])
            t2v = const.tile([1, P], f8)
            nc.scalar.dma_start(out=t2v, in_=t2v_d[:, :])
            t1 = tb[:, 0:PC]
            s2 = tb[:, PC:2 * PC]
            nc.sync.dma_start(out=x8c0[1], in_=x8_d[0, :, 4 * 784:8 * 784])
            nc.sync.dma_start(
                out=w18b, in_=w18_d[1:4, :, :].rearrange("o p s -> p o s"))
            wqk8 = const.tile([128, 8, 512], f8)
            nc.sync.dma_start(out=wqk8, in_=wqk8_d[:, :, :])
            # pair-1 x8 stays on sync BEHIND the pair-0 critical transfers;
            # a second queue would steal DMA bandwidth from them
            emit_x8_dma(1)

            wvt = const.tile([128, PC, P], f8)
            pos = const.tile([128, PC, N], bf16)
            w3t = const.tile([128, PC, CIN], f8)
            ones_sb = const.tile([128, 128], bf16)
            nc.gpsimd.memset(ones_sb, 1.0)
            from concourse.masks import make_identity
            identb = const.tile([128, 128], bf16)
            make_identity(nc, identb)

            def emit_late_weights():
                nc.sync.dma_start(out=wvt, in_=wvt_d[:, :, :])
                nc.sync.dma_start(out=pos, in_=pos_d[:, :, :])
                nc.sync.dma_start(out=w3t, in_=w3t_d[:, :, :])

            # ---------------- block emitters ----------------
            def conv1_block(p, oc):
                Sp = S[p]
                if oc == 0:
                    # cols = j*256 + n (per-image padded to 256 so the vT
                    # stationary slices meet the DR 16B step alignment)
                    Sp['h18'] = h18p.tile([128, 2, 2, 512], f8,
                                          name=f"h18_{p}", tag="h18")
                cps = ps_mm.tile([128, 512], f32, name="cps", tag="mm")
                for kc in range(KC2):
                    nc.tensor.matmul(
                        cps[:, :N2],
                        w18sl(oc, kc),
                        x8sl(p, kc),
                        start=(kc == 0), stop=(kc == KC2 - 1),
                        perf_mode=DR,
                    )
                # h18 = relu(cps/8 + t1), straight to fp8
                nc.scalar.activation(
                    Sp['h18'][:, oc // 2, oc % 2, :].rearrange(
                        "p (j w) -> p j w", j=2)[:, :, :N],
                    cps[:, :N2].rearrange("p (j n) -> p j n", j=2),
                    Relu, bias=t1[:, oc:oc + 1], scale=1.0 / WS)

            def qk_block(p, oc):
                Sp = S[p]
                if oc == 0:
                    Sp['q'] = qkp.tile([128, PC, 2, N], bf16,
                                       name=f"q_{p}", tag="q")
                    Sp['k'] = qkp.tile([128, PC, 2, N], bf16,
                                       name=f"k_{p}", tag="k")
                qps = ps_mm.tile([128, 512], f32, name="qps", tag="mm")
                for kc in range(2):
                    nc.tensor.matmul(
                        qps[:, :],
                        wqk8[:, oc, kc * 256:(kc + 1) * 256].rearrange(
                            "p (s m) -> p s m", s=2),
                        Sp['h18'][:, kc, :, :],
                        start=(kc == 0), stop=(kc == 1),
                        perf_mode=DR,
                    )
                dst = Sp['q'] if oc < PC else Sp['k']
                c4 = oc % PC
                qv = qps[:, :].rearrange("p (j w) -> p j w", j=2)[:, :, :N]
                if oc % 2 == 0:
                    nc.scalar.activation(dst[:, c4, :, :], qv,
                                         Copy, scale=1.0 / WS)
                else:
                    nc.vector.tensor_scalar_mul(dst[:, c4, :, :], qv,
                                                1.0 / WS)

            def vT_block(p, j):
                Sp = S[p]
                # vT holds 8*v (wvt pre-scaled); undone in the h2 activation
                vT = attp.tile([128, 2, P], f8, name=f"vT_{p}_{j}", tag="vT")
                expT = attp.tile([128, 2, N], bf16, name=f"eT_{p}_{j}",
                                 tag="expT")
                attnT = attp.tile([128, 2, N], f8, name=f"aT_{p}_{j}",
                                  tag="attnT")
                Sp[f'vT{j}'], Sp[f'expT{j}'], Sp[f'attnT{j}'] = vT, expT, attnT
                # rows 68.. of the second m-chunk stay zero (m=196..255 pad);
                # vT pad rows are garbage-filled, so zero them too (fp8 NaN
                # times attn 0 would poison the aout contraction).
                # Pad row 96 (m=224) carries the h2 bias: attn=1, vT=8*t2/s2,
                # so the aout matmul itself adds t2 and the h2 drain is a
                # bias-free relu*scale that either engine can run.
                nc.gpsimd.memset(attnT[64:128, 1, :], 0.0)
                nc.gpsimd.memset(attnT[96:97, 1, :], 1.0)
                # rows 64:68 are re-written by the vps copy below
                nc.gpsimd.memset(vT[64:128, 1, :], 0.0)
                nc.vector.tensor_copy(vT[96:97, 1, :], t2v[0:1, :])
                for mi, (m0, msz) in enumerate(NCHUNKS):
                    vps = ps_mm.tile([128, 512], f32, name="vps", tag="mm")
                    for i in range(2):
                        nc.tensor.matmul(
                            vps[:msz, :],
                            Sp['h18'][:, i, :,
                                      j * 256 + m0:j * 256 + m0 + msz],
                            wvt[:, 2 * i:2 * i + 2, :],
                            start=(i == 0), stop=(i == 1),
                            perf_mode=DR,
                        )
                    nc.vector.tensor_copy(vT[:msz, mi, :], vps[:msz, :])

            def sT_block(p, j, mi):
                Sp = S[p]
                m0, msz = NCHUNKS[mi]
                q, k = Sp['q'], Sp['k']
                lps = ps_sm.tile([128, 256], f32, name="lps", tag="small")
                # scores transposed: sT[m, n] = sum_c k[c,m] q[c,n]
                #                             + sum_c q[c,m] pos[c,n]
                for pc in range(PC):
                    nc.tensor.matmul(
                        lps[:msz, :N],
                        k[:, pc, j, m0:m0 + msz],
                        q[:, pc, j, :],
                        start=(pc == 0), stop=False,
                    )
                for pc in range(PC):
                    nc.tensor.matmul(
                        lps[:msz, :N],
                        q[:, pc, j, m0:m0 + msz],
                        pos[:, pc, :],
                        start=False, stop=(pc == PC - 1),
                    )
                # exp (no max subtraction: logits O(40) max, finite in fp32,
                # and bf16 holds e^40 fine)
                nc.scalar.activation(Sp[f'expT{j}'][:msz, mi, :],
                                     lps[:msz, :N], Exp)

            def softsum_block(p, j):
                Sp = S[p]
                expT = Sp[f'expT{j}']
                spsum = ps_sm.tile([1, 256], f32, name="spsum", tag="small")
                for mi, (m0, msz) in enumerate(NCHUNKS):
                    nc.tensor.matmul(
                        spsum[:1, :N],
                        ones_sb[:msz, 0:1],
                        expT[:msz, mi, :],
                        start=(mi == 0), stop=(mi == 1),
                    )
                Sp[f'spsum{j}'] = spsum

            def softnorm_block(p, j):
                Sp = S[p]
                rinv32 = attp.tile([1, N], f32, name=f"rinv32_{p}_{j}",
                                   tag="rinv32")
                nc.vector.reciprocal_approx_fast(rinv32[:1, :],
                                                 Sp[f'spsum{j}'][:1, :N])
                rinv = attp.tile([1, N], bf16, name=f"rinv_{p}_{j}",
                                 tag="rinv")
                nc.vector.tensor_copy(rinv[:1, :], rinv32[:1, :])
                rps = ps_sm.tile([128, 256], f32, name="rps", tag="small")
                nc.tensor.matmul(rps[:, :N], ones_sb[0:1, :], rinv[:1, :],
                                 start=True, stop=True)
                expT, attnT = Sp[f'expT{j}'], Sp[f'attnT{j}']
                for mi, (m0, msz) in enumerate(NCHUNKS):
                    nc.vector.tensor_mul(attnT[:msz, mi, :],
                                         expT[:msz, mi, :], rps[:msz, :N])

            def aout_block(p, j):
                Sp = S[p]
                if j == 0:
                    Sp['h2'] = h2p.tile([128, 2, 2, N2], f8,
                                        name=f"h2_{p}", tag="h2")
                vT, attnT = Sp[f'vT{j}'], Sp[f'attnT{j}']
                # two c4 chunks share one PSUM bank: halves the allocation
                # count so the drain rotation runs two ocs deep
                for c2 in range(2):
                    aps = ps_sm.tile([128, 512], f32, name="aps", tag="small")
                    for d in range(2):
                        c4 = 2 * c2 + d
                        nc.tensor.matmul(
                            aps[:, 256 * d:256 * d + N],
                            vT[:, :, c4 * 128:(c4 + 1) * 128],
                            attnT[:, :, :],
                            start=True, stop=True,
                            perf_mode=DR, skip_group_check=True,
                        )
                    # h2 = relu((s2/8)*(aps+8*t2/s2)) = relu(s2/8*aps + t2)
                    # (t2 comes via the pad row; s2 pre-divided, >0)
                    for d in range(2):
                        c4 = 2 * c2 + d
                        dst = Sp['h2'][:, c4 // 2, c4 % 2,
                                       j * N:(j + 1) * N]
                        src = aps[:, 256 * d:256 * d + N]
                        if c4 % 2 == 0:
                            nc.scalar.activation(dst, src, Relu,
                                                 scale=s2[:, c4:c4 + 1])
                        else:
                            nc.vector.tensor_scalar(
                                dst, src, 0.0, s2[:, c4:c4 + 1],
                                op0=mybir.AluOpType.max,
                                op1=mybir.AluOpType.mult)

            def conv3_block(p, k4, final=False):
                Sp = S[p]
                y_sb = outp.tile([128, 4, N2], bf16, name="y_sb", tag="y_sb")
                # 2-oc groups with the two bf16 identity (residual) matmuls
                # back-to-back: half the PE DR<->bf16 mode transitions
                for h in range(2):
                    oc0 = 4 * k4 + 2 * h
                    opsl = [ps_mm.tile([128, 512], f32, name="ops", tag="mm")
                            for _ in range(2)]
                    for d in range(2):
                        for ch in range(2):
                            nc.tensor.matmul(
                                opsl[d][:, :N2],
                                w3t[:, 2 * ch:2 * ch + 2,
                                    (oc0 + d) * 128:(oc0 + d + 1) * 128],
                                Sp['h2'][:, ch, :, :],
                                start=(ch == 0), stop=False,
                                perf_mode=DR, skip_group_check=True,
                            )
                    for d in range(2):
                        # residual + t3 folded in via identity matmul on xr
                        nc.tensor.matmul(opsl[d][:, :N2], identb[:, :],
                                         Sp['xr'][:, oc0 + d, :],
                                         start=False, stop=True,
                                         skip_group_check=True)
                    for d in range(2):
                        i4 = 2 * h + d
                        if d == 0:
                            nc.scalar.activation(y_sb[:, i4, :],
                                                 opsl[d][:, :N2], Relu)
                        else:
                            nc.vector.tensor_scalar_max(y_sb[:, i4, :],
                                                        opsl[d][:, :N2], 0.0)
                if final:
                    # drain the last stores per-2-oc on two queues so they
                    # overlap the remaining relus
                    for h in range(2):
                        eng = nc.sync if h == 0 else nc.gpsimd
                        nc_eng = eng
                        nc_eng.dma_start(
                            out=y_d[p, :, 4 * k4 + 2 * h:4 * k4 + 2 * h + 2,
                                    :],
                            in_=y_sb[:, 2 * h:2 * h + 2, :])
                else:
                    nc.gpsimd.dma_start(
                        out=y_d[p, :, 4 * k4:4 * k4 + 4, :],
                        in_=y_sb[:, :, :])

            # ---------------- pipeline driver ----------------
            def A_blocks(p):
                return ([lambda p=p, oc=oc: conv1_block(p, oc)
                         for oc in range(PC)] +
                        [lambda p=p, oc=oc: qk_block(p, oc)
                         for oc in range(2 * PC)])

            def B_blocks(p, final=False):
                # the two images' chains are independent; interleaving them
                # j0/j1 gives every dependent step a full block of slack
                out = []
                for step in (vT_block,
                             lambda p, j: sT_block(p, j, 0),
                             lambda p, j: sT_block(p, j, 1),
                             softsum_block, softnorm_block, aout_block):
                    for j in range(2):
                        out.append(lambda p=p, j=j, s=step: s(p, j))
                out += [lambda p=p, k=k: conv3_block(p, k, final)
                        for k in range(4)]
                return out

            def interleave(Bl, Al):
                nB, nA = len(Bl), len(Al)
                ai = 0
                for bi, b in enumerate(Bl):
                    b()
                    target = ((bi + 1) * nA) // nB
                    while ai < target:
                        Al[ai]()
                        ai += 1
                while ai < nA:
                    Al[ai]()
                    ai += 1

            prevB = None
            for p in range(NPAIR):
                A = A_blocks(p)
                if prevB is None:
                    for idx, a in enumerate(A):
                        a()
                        if idx == 1:
                            emit_late_weights()
                            # xr(0) on sync too: gpsimd would start it
                            # immediately and compete with pair-0 transfers
                            emit_xr_dma(0, eng=nc.sync)
                else:
                    # prefetch ahead of this iteration's y stores
                    if p + 1 < NPAIR:
                        emit_x8_dma(p + 1)
                    emit_xr_dma(p)
                    interleave(prevB, A)
                prevB = B_blocks(p, final=(p == NPAIR - 1))
            for b in prevB:
                b()

    nc.compile()
    return nc


def _prep_inputs(x, w1, g1, b1, m1, v1, wqkv, rel_h, rel_w,
                 g2, b2, m2, v2, w3, g3, b3, m3, v3):
    f = np.float32
    bf = ml_dtypes.bfloat16
    f8 = ml_dtypes.float8_e4m3
    s1 = (g1 / np.sqrt(v1 + EPS)).astype(f)
    t1 = (b1 - m1 * s1).astype(f)
    s2 = (g2 / np.sqrt(v2 + EPS)).astype(f)
    t2 = (b2 - m2 * s2).astype(f)
    s3 = (g3 / np.sqrt(v3 + EPS)).astype(f)
    t3 = (b3 - m3 * s3).astype(f)

    # conv1 weights: fold s1, pre-scale x8, DR stationary layout
    # [ocb, p, kc2*256 + sub*128 + ocm]; cin = kc2*256 + sub*128 + p
    w1p = (w1 * s1[:, None] * WS).astype(f)               # [512, 2048]
    w18 = w1p.reshape(PC, 128, KC2, 2, 128).transpose(0, 4, 2, 3, 1)
    w18 = np.ascontiguousarray(w18.reshape(PC, 128, KC2 * 256)).astype(f8)

    # qk weights: [p, ocb(8), kc2*256 + sub*128 + ocm]
    wqk = (wqkv[:2 * P] * WS).astype(f)                   # [1024, 512]
    wqk8 = wqk.reshape(8, 128, 2, 2, 128).transpose(4, 0, 2, 3, 1)
    wqk8 = np.ascontiguousarray(wqk8.reshape(128, 8, 512)).astype(f8)

    wv = (wqkv[2 * P:] * WS).astype(f)                    # [512, 512]
    wvt = np.ascontiguousarray(
        wv.T.reshape(PC, 128, P).transpose(1, 0, 2)).astype(f8)
    w3p = (w3 * s3[:, None]).astype(f)                    # [2048, 512]
    w3t = np.ascontiguousarray(
        w3p.T.reshape(PC, 128, CIN).transpose(1, 0, 2)).astype(f8)
    pos = (rel_h + rel_w).reshape(P, N).astype(f)
    pos = np.ascontiguousarray(
        pos.reshape(PC, 128, N).transpose(1, 0, 2)).astype(bf)

    tb = np.concatenate([t1.reshape(PC, 128).T,
                         (s2 / WS).reshape(PC, 128).T], axis=1)
    tb = np.ascontiguousarray(tb, f)
    t2v = np.ascontiguousarray((WS * t2 / s2).reshape(1, P)).astype(f8)

    shared = dict(w18=w18, wqk8=wqk8, wvt=wvt, w3t=w3t, pos=pos, tb=tb,
                  t2v=t2v)

    xf = np.asarray(x, f)
    in_maps = []
    for c in range(NCORES):
        xc = xf[c * BPC:(c + 1) * BPC].reshape(BPC, CIN, N)
        # fp8 copy, partition-major DR layout:
        # [pair, p, kc2*784 + sub*392 + j*196 + n]
        x8 = xc.reshape(NPAIR, 2, KC2, 2, 128, N).transpose(0, 4, 2, 3, 1, 5)
        x8 = np.ascontiguousarray(
            x8.reshape(NPAIR, 128, KC2 * 784)).astype(f8)
        # bf16 residual copy with t3: [pair, p, kc1*392 + j*196 + n]
        xr = xc + t3[None, :, None]
        xr = xr.reshape(NPAIR, 2, KC1, 128, N).transpose(0, 3, 2, 1, 4)
        xr = np.ascontiguousarray(
            xr.reshape(NPAIR, 128, KC1 * N2)).astype(bf)
        in_maps.append(dict(shared, x8=x8, xr=xr))
    return in_maps


def _run(in_maps, trace=False, tmpdir=None):
    from concourse.bass_utils import run_bass_kernel_spmd
    if "nc" not in _CACHE:
        _CACHE["nc"] = _build()
    nc = _CACHE["nc"]
    return run_bass_kernel_spmd(nc, in_maps, core_ids=list(range(NCORES)),
                                trace=trace, tmpdir=tmpdir)


def _post(res):
    out = np.empty((B, CIN, H, W), np.float32)
    for c in range(NCORES):
        # y: [pair, p, kc1, j*196 + n]
        yc = res.results[c]["y"].astype(np.float32).reshape(
            NPAIR, 128, KC1, 2, N)
        out[c * BPC:(c + 1) * BPC] = yc.transpose(0, 3, 2, 1, 4).reshape(
            BPC, CIN, H, W)
    return out


def kernel(**inputs):
    in_maps = _prep_inputs(**inputs)
    res = _run(in_maps)
    return _post(res)


# revision 36
# speedup vs baseline: 1.0852x; 1.0579x over previous
"""Trainium2 Bass kernel for the MHSA bottleneck block.

Contract: kernel(**inputs) takes the FULL unsharded inputs (as produced by
setup_inputs()) and returns the FULL [64, 2048, 14, 14] float32 output.
Internally shards data-parallel over batch: 8 images per NeuronCore, 8 cores.

v5: fp8 DoubleRow conv1 + q/k projection (on top of v3's fp8
value/attention/conv3), identity-matmul residual with host-prefolded
xr = x + t3 (bare-relu epilogue), x8-prescaled fp8 weights (undone via
activation scales), and partition-major DRAM layouts so DMA descriptors move
3-12KB contiguous runs per partition instead of 784B (the v4 startup was
DMA-packet-bound at ~40GB/s).
"""
import sys

sys.path.insert(0, '/opt/trn_rl_repo')

import numpy as np
import ml_dtypes

# Problem constants (hardcoded per the harness contract).
B, CIN, P, H, W = 64, 2048, 512, 14, 14
EPS = 1e-5
N = H * W            # 196 pixels
NCORES = 8
BPC = B // NCORES    # 8 images per core
NPAIR = BPC // 2     # 4 image pairs per core
KC1 = CIN // 128     # 16 input-channel chunks (bf16 view) for residual/y
KC2 = CIN // 256     # 8 DoubleRow input-channel chunks for conv1
PC = P // 128        # 4 chunks of the 512-dim
N2 = 2 * N           # 392 = free dim for image-pair matmuls
WS = 8.0             # host-side fp8 weight pre-scale (undone on-chip)

# m chunking of the 196-pixel dim: 128 + 68
NCHUNKS = [(0, 128), (128, 68)]

_CACHE = {}


def _build():
    import concourse.bass as bass  # noqa: F401
    import concourse.mybir as mybir
    import concourse.tile as tile
    from concourse import bacc

    f32 = mybir.dt.float32
    bf16 = mybir.dt.bfloat16
    f8 = mybir.dt.float8e4

    DR = mybir.MatmulPerfMode.DoubleRow

    nc = bacc.Bacc(None, target_bir_lowering=False, debug=False)

    # fp8 x, partition-major: [pair, p, kc2*784 + sub*392 + j*196 + n]
    x8_d = nc.declare_dram_parameter("x8", [NPAIR, 128, KC2 * 784], f8,
                                     isOutput=False)
    # bf16 x with t3 folded in: [pair, p, kc1*392 + j*196 + n]
    xr_d = nc.declare_dram_parameter("xr", [NPAIR, 128, KC1 * N2], bf16,
                                     isOutput=False)
    # conv1 weights fp8 x8-scaled: [ocb, p, kc2*256 + sub*128 + ocm]
    w18_d = nc.declare_dram_parameter("w18", [PC, 128, KC2 * 256], f8,
                                      isOutput=False)
    # qk weights fp8 x8-scaled: [p, ocb, kc2*256 + sub*128 + ocm]
    wqk8_d = nc.declare_dram_parameter("wqk8", [128, 8, 512], f8,
                                       isOutput=False)
    # value weights fp8 x8-scaled: [p, pc, P]
    wvt_d = nc.declare_dram_parameter("wvt", [128, PC, P], f8, isOutput=False)
    # conv3 weights fp8 (unscaled): [p, pc, CIN]
    w3t_d = nc.declare_dram_parameter("w3t", [128, PC, CIN], f8,
                                      isOutput=False)
    pos_d = nc.declare_dram_parameter("pos", [128, PC, N], bf16,
                                      isOutput=False)
    # packed per-channel bias/scale vectors: t1 | s2/8
    tb_d = nc.declare_dram_parameter("tb", [128, 2 * PC], f32, isOutput=False)
    # 8*t2/s2 as a bf16 row, injected via the attention pad row
    t2v_d = nc.declare_dram_parameter("t2v", [1, P], bf16, isOutput=False)
    # output, partition-major: [pair, p, kc1, j*196 + n]
    y_d = nc.declare_dram_parameter("y", [NPAIR, 128, KC1, N2], bf16,
                                    isOutput=True)

    with tile.TileContext(nc) as tc:
        with (
            tc.tile_pool(name="const", bufs=1) as const,
            tc.tile_pool(name="x8p", bufs=3) as x8p,
            tc.tile_pool(name="xrp", bufs=2) as xrp,
            tc.tile_pool(name="h18p", bufs=2) as h18p,
            tc.tile_pool(name="qkp", bufs=2) as qkp,
            tc.tile_pool(name="h2p", bufs=2) as h2p,
            tc.tile_pool(name="attp", bufs=2) as attp,
            tc.tile_pool(name="outp", bufs=4) as outp,
            tc.tile_pool(name="ps_mm", bufs=5, space="PSUM") as ps_mm,
            tc.tile_pool(name="ps_sm", bufs=3, space="PSUM") as ps_sm,
        ):
            Exp = mybir.ActivationFunctionType.Exp
            Relu = mybir.ActivationFunctionType.Relu
            Copy = mybir.ActivationFunctionType.Copy

            S = [dict() for _ in range(NPAIR)]

            # ---------------- DMA emitters ----------------
            # Each dma_start costs ~700ns of serial issue time on its queue,
            # so startup batches transfers into few calls and spreads issues
            # across queues (tb/t2v on vector, pair-1 x8 on scalar).
            def emit_x8_dma(p, eng=None):
                t = x8p.tile([128, KC2, 2, N2], f8, name=f"x8_{p}", tag="x8")
                S[p]['x8'] = t
                (eng or nc.sync).dma_start(out=t, in_=x8_d[p, :, :])

            def emit_xr_dma(p, eng=None):
                t = xrp.tile([128, KC1, N2], bf16, name=f"xr_{p}", tag="xr")
                S[p]['xr'] = t
                (eng or nc.gpsimd).dma_start(out=t, in_=xr_d[p, :, :])

            # pair-0 x8 in two halves (kc0-3 | kc4-7): 3.1KB runs per
            # partition keep the DMA out of its per-packet-bound regime
            x8c0 = [const.tile([128, 4, 2, N2], f8, name="x8c0a"),
                    const.tile([128, 4, 2, N2], f8, name="x8c0b")]
            w18a = const.tile([128, KC2 * 256], f8, name="w18a")
            w18b = const.tile([128, 3, KC2 * 256], f8, name="w18b")

            def w18sl(oc, kc):
                t = w18a if oc == 0 else w18b[:, oc - 1, :]
                return t[:, kc * 256:(kc + 1) * 256].rearrange(
                    "p (s m) -> p s m", s=2)

            def x8sl(p, kc):
                if p > 0:
                    return S[p]['x8'][:, kc, :, :]
                return x8c0[kc // 4][:, kc % 4, :, :]

            nc.sync.dma_start(out=w18a, in_=w18_d[0, :, :])
            nc.sync.dma_start(out=x8c0[0], in_=x8_d[0, :, 0:4 * 784])
            tb = const.tile([128, 2 * PC], f32)
            nc.scalar.dma_start(out=tb, in_=tb_d[:, :])
            t2v = const.tile([1, P], bf16)
            nc.scalar.dma_start(out=t2v, in_=t2v_d[:, :])
            t1 = tb[:, 0:PC]
            s2 = tb[:, PC:2 * PC]
            nc.sync.dma_start(out=x8c0[1], in_=x8_d[0, :, 4 * 784:8 * 784])
            nc.sync.dma_start(
                out=w18b, in_=w18_d[1:4, :, :].rearrange("o p s -> p o s"))
            wqk8 = const.tile([128, 8, 512], f8)
            nc.sync.dma_start(out=wqk8, in_=wqk8_d[:, :, :])
            # pair-1 x8 stays on sync BEHIND the pair-0 critical transfers;
            # a second queue would steal DMA bandwidth from them
            emit_x8_dma(1)

            wvt = const.tile([128, PC, P], f8)
            pos = const.tile([128, PC, N], bf16)
            w3t = const.tile([128, PC, CIN], f8)
            ones_sb = const.tile([128, 128], bf16)
            nc.gpsimd.memset(ones_sb, 1.0)
            from concourse.masks import make_identity
            identb = const.tile([128, 128], bf16)
            make_identity(nc, identb)

            def emit_late_weights():
                nc.sync.dma_start(out=wvt, in_=wvt_d[:, :, :])
                nc.sync.dma_start(out=pos, in_=pos_d[:, :, :])
                nc.sync.dma_start(out=w3t, in_=w3t_d[:, :, :])

            # ---------------- block emitters ----------------
            def conv1_block(p, oc):
                Sp = S[p]
                if oc == 0:
                    # cols = j*256 + n (per-image padded to 256 so the vT
                    # stationary slices meet the DR 16B step alignment)
                    Sp['h18'] = h18p.tile([128, 2, 2, 512], f8,
                                          name=f"h18_{p}", tag="h18")
                cps = ps_mm.tile([128, 512], f32, name="cps", tag="mm")
                for kc in range(KC2):
                    nc.tensor.matmul(
                        cps[:, :N2],
                        w18sl(oc, kc),
                        x8sl(p, kc),
                        start=(kc == 0), stop=(kc == KC2 - 1),
                        perf_mode=DR,
                    )
                # h18 = relu(cps/8 + t1), straight to fp8
                nc.scalar.activation(
                    Sp['h18'][:, oc // 2, oc % 2, :].rearrange(
                        "p (j w) -> p j w", j=2)[:, :, :N],
                    cps[:, :N2].rearrange("p (j n) -> p j n", j=2),
                    Relu, bias=t1[:, oc:oc + 1], scale=1.0 / WS)

            def qk_block(p, oc2):
                # two oc blocks with interleaved accumulation chains so each
                # chain's (serial) DR weight load hides under the other's
                # stream
                Sp = S[p]
                if oc2 == 0:
                    Sp['q'] = qkp.tile([128, PC, 2, N], bf16,
                                       name=f"q_{p}", tag="q")
                    Sp['k'] = qkp.tile([128, PC, 2, N], bf16,
                                       name=f"k_{p}", tag="k")
                qpsl = [ps_mm.tile([128, 512], f32, name="qps", tag="mm")
                        for _ in range(2)]
                for kc in range(2):
                    for d in range(2):
                        nc.tensor.matmul(
                            qpsl[d][:, :],
                            wqk8[:, 2 * oc2 + d,
                                 kc * 256:(kc + 1) * 256].rearrange(
                                "p (s m) -> p s m", s=2),
                            Sp['h18'][:, kc, :, :],
                            start=(kc == 0), stop=(kc == 1),
                            perf_mode=DR, skip_group_check=True,
                        )
                for d in range(2):
                    oc = 2 * oc2 + d
                    dst = Sp['q'] if oc < PC else Sp['k']
                    c4 = oc % PC
                    qv = qpsl[d][:, :].rearrange(
                        "p (j w) -> p j w", j=2)[:, :, :N]
                    if oc % 2 == 0:
                        nc.scalar.activation(dst[:, c4, :, :], qv,
                                             Copy, scale=1.0 / WS)
                    else:
                        nc.vector.tensor_scalar_mul(dst[:, c4, :, :], qv,
                                                    1.0 / WS)

            def vT_block(p, j):
                Sp = S[p]
                # vT holds 8*v (wvt pre-scaled); undone in the h2 activation.
                # bf16 vT/attnT: non-DR aout matmuls get overlapped weight
                # loads, and the attention weights skip fp8 quantization.
                vT = attp.tile([128, 2, P], bf16, name=f"vT_{p}_{j}",
                               tag="vT")
                expT = attp.tile([128, 2, N], bf16, name=f"eT_{p}_{j}",
                                 tag="expT")
                attnT = attp.tile([128, 2, N], bf16, name=f"aT_{p}_{j}",
                                  tag="attnT")
                Sp[f'vT{j}'], Sp[f'expT{j}'], Sp[f'attnT{j}'] = vT, expT, attnT
                # rows 68.. of the second m-chunk stay zero (m=196..255 pad);
                # vT pad rows are garbage-filled, so zero them too (fp8 NaN
                # times attn 0 would poison the aout contraction).
                # Pad row 96 (m=224) carries the h2 bias: attn=1, vT=8*t2/s2,
                # so the aout matmul itself adds t2 and the h2 drain is a
                # bias-free relu*scale that either engine can run.
                nc.gpsimd.memset(attnT[64:128, 1, :], 0.0)
                nc.gpsimd.memset(attnT[96:97, 1, :], 1.0)
                # rows 64:68 are re-written by the vps copy below
                nc.gpsimd.memset(vT[64:128, 1, :], 0.0)
                nc.vector.tensor_copy(vT[96:97, 1, :], t2v[0:1, :])
                for mi, (m0, msz) in enumerate(NCHUNKS):
                    vps = ps_mm.tile([128, 512], f32, name="vps", tag="mm")
                    for i in range(2):
                        nc.tensor.matmul(
                            vps[:msz, :],
                            Sp['h18'][:, i, :,
                                      j * 256 + m0:j * 256 + m0 + msz],
                            wvt[:, 2 * i:2 * i + 2, :],
                            start=(i == 0), stop=(i == 1),
                            perf_mode=DR,
                        )
                    nc.vector.tensor_copy(vT[:msz, mi, :], vps[:msz, :])

            def sT_block(p, j, mi):
                Sp = S[p]
                m0, msz = NCHUNKS[mi]
                q, k = Sp['q'], Sp['k']
                lps = ps_sm.tile([128, 256], f32, name="lps", tag="small")
                # scores transposed: sT[m, n] = sum_c k[c,m] q[c,n]
                #                             + sum_c q[c,m] pos[c,n]
                for pc in range(PC):
                    nc.tensor.matmul(
                        lps[:msz, :N],
                        k[:, pc, j, m0:m0 + msz],
                        q[:, pc, j, :],
                        start=(pc == 0), stop=False,
                    )
                for pc in range(PC):
                    nc.tensor.matmul(
                        lps[:msz, :N],
                        q[:, pc, j, m0:m0 + msz],
                        pos[:, pc, :],
                        start=False, stop=(pc == PC - 1),
                    )
                # exp (no max subtraction: logits O(40) max, finite in fp32,
                # and bf16 holds e^40 fine)
                nc.scalar.activation(Sp[f'expT{j}'][:msz, mi, :],
                                     lps[:msz, :N], Exp)

            def softsum_block(p, j):
                Sp = S[p]
                expT = Sp[f'expT{j}']
                spsum = ps_sm.tile([1, 256], f32, name="spsum", tag="small")
                for mi, (m0, msz) in enumerate(NCHUNKS):
                    nc.tensor.matmul(
                        spsum[:1, :N],
                        ones_sb[:msz, 0:1],
                        expT[:msz, mi, :],
                        start=(mi == 0), stop=(mi == 1),
                    )
                Sp[f'spsum{j}'] = spsum

            def softnorm_block(p, j):
                Sp = S[p]
                rinv32 = attp.tile([1, N], f32, name=f"rinv32_{p}_{j}",
                                   tag="rinv32")
                nc.vector.reciprocal_approx_fast(rinv32[:1, :],
                                                 Sp[f'spsum{j}'][:1, :N])
                rinv = attp.tile([1, N], bf16, name=f"rinv_{p}_{j}",
                                 tag="rinv")
                nc.vector.tensor_copy(rinv[:1, :], rinv32[:1, :])
                rps = ps_sm.tile([128, 256], f32, name="rps", tag="small")
                nc.tensor.matmul(rps[:, :N], ones_sb[0:1, :], rinv[:1, :],
                                 start=True, stop=True)
                expT, attnT = Sp[f'expT{j}'], Sp[f'attnT{j}']
                for mi, (m0, msz) in enumerate(NCHUNKS):
                    nc.vector.tensor_mul(attnT[:msz, mi, :],
                                         expT[:msz, mi, :], rps[:msz, :N])

            def aout_block(p, j):
                Sp = S[p]
                if j == 0:
                    Sp['h2'] = h2p.tile([128, 2, 2, N2], f8,
                                        name=f"h2_{p}", tag="h2")
                vT, attnT = Sp[f'vT{j}'], Sp[f'attnT{j}']
                # two c4 chunks share one PSUM bank: halves the allocation
                # count so the drain rotation runs two ocs deep
                for c2 in range(2):
                    aps = ps_sm.tile([128, 512], f32, name="aps", tag="small")
                    for d in range(2):
                        c4 = 2 * c2 + d
                        for mi in range(2):
                            nc.tensor.matmul(
                                aps[:, 256 * d:256 * d + N],
                                vT[:, mi, c4 * 128:(c4 + 1) * 128],
                                attnT[:, mi, :],
                                start=(mi == 0), stop=(mi == 1),
                                skip_group_check=True,
                            )
                    # h2 = relu((s2/8)*(aps+8*t2/s2)) = relu(s2/8*aps + t2)
                    # (t2 comes via the pad row; s2 pre-divided, >0)
                    for d in range(2):
                        c4 = 2 * c2 + d
                        dst = Sp['h2'][:, c4 // 2, c4 % 2,
                                       j * N:(j + 1) * N]
                        src = aps[:, 256 * d:256 * d + N]
                        if c4 % 2 == 0:
                            nc.scalar.activation(dst, src, Relu,
                                                 scale=s2[:, c4:c4 + 1])
                        else:
                            nc.vector.tensor_scalar(
                                dst, src, 0.0, s2[:, c4:c4 + 1],
                                op0=mybir.AluOpType.max,
                                op1=mybir.AluOpType.mult)

            def conv3_block(p, k4, final=False):
                Sp = S[p]
                y_sb = outp.tile([128, 4, N2], bf16, name="y_sb", tag="y_sb")
                # 2-oc groups with the two bf16 identity (residual) matmuls
                # back-to-back: half the PE DR<->bf16 mode transitions
                for h in range(2):
                    oc0 = 4 * k4 + 2 * h
                    opsl = [ps_mm.tile([128, 512], f32, name="ops", tag="mm")
                            for _ in range(2)]
                    for ch in range(2):
                        for d in range(2):
                            nc.tensor.matmul(
                                opsl[d][:, :N2],
                                w3t[:, 2 * ch:2 * ch + 2,
                                    (oc0 + d) * 128:(oc0 + d + 1) * 128],
                                Sp['h2'][:, ch, :, :],
                                start=(ch == 0), stop=False,
                                perf_mode=DR, skip_group_check=True,
                            )
                    for d in range(2):
                        # residual + t3 folded in via identity matmul on xr
                        nc.tensor.matmul(opsl[d][:, :N2], identb[:, :],
                                         Sp['xr'][:, oc0 + d, :],
                                         start=False, stop=True,
                                         skip_group_check=True)
                    for d in range(2):
                        i4 = 2 * h + d
                        if d == 0:
                            nc.scalar.activation(y_sb[:, i4, :],
                                                 opsl[d][:, :N2], Relu)
                        else:
                            nc.vector.tensor_scalar_max(y_sb[:, i4, :],
                                                        opsl[d][:, :N2], 0.0)
                if final:
                    # drain the last stores per-2-oc on two queues so they
                    # overlap the remaining relus
                    for h in range(2):
                        eng = nc.sync if h == 0 else nc.gpsimd
                        nc_eng = eng
                        nc_eng.dma_start(
                            out=y_d[p, :, 4 * k4 + 2 * h:4 * k4 + 2 * h + 2,
                                    :],
                            in_=y_sb[:, 2 * h:2 * h + 2, :])
                else:
                    nc.gpsimd.dma_start(
                        out=y_d[p, :, 4 * k4:4 * k4 + 4, :],
                        in_=y_sb[:, :, :])

            # ---------------- pipeline driver ----------------
            def A_blocks(p):
                return ([lambda p=p, oc=oc: conv1_block(p, oc)
                         for oc in range(PC)] +
                        [lambda p=p, oc2=oc2: qk_block(p, oc2)
                         for oc2 in range(PC)])

            def B_blocks(p, final=False):
                # the two images' chains are independent; interleaving them
                # j0/j1 gives every dependent step a full block of slack
                out = []
                for step in (vT_block,
                             lambda p, j: sT_block(p, j, 0),
                             lambda p, j: sT_block(p, j, 1),
                             softsum_block, softnorm_block, aout_block):
                    for j in range(2):
                        out.append(lambda p=p, j=j, s=step: s(p, j))
                out += [lambda p=p, k=k: conv3_block(p, k, final)
                        for k in range(4)]
                return out

            def interleave(Bl, Al):
                nB, nA = len(Bl), len(Al)
                ai = 0
                for bi, b in enumerate(Bl):
                    b()
                    target = ((bi + 1) * nA) // nB
                    while ai < target:
                        Al[ai]()
                        ai += 1
                while ai < nA:
                    Al[ai]()
                    ai += 1

            prevB = None
            for p in range(NPAIR):
                A = A_blocks(p)
                if prevB is None:
                    for idx, a in enumerate(A):
                        a()
                        if idx == 1:
                            emit_late_weights()
                            # xr(0) on sync too: gpsimd would start it
                            # immediately and compete with pair-0 transfers
                            emit_xr_dma(0, eng=nc.sync)
                else:
                    # prefetch ahead of this iteration's y stores
                    if p + 1 < NPAIR:
                        emit_x8_dma(p + 1)
                    emit_xr_dma(p)
                    interleave(prevB, A)
                prevB = B_blocks(p, final=(p == NPAIR - 1))
            for b in prevB:
                b()

    nc.compile()
    return nc


def _prep_inputs(x, w1, g1, b1, m1, v1, wqkv, rel_h, rel_w,
                 g2, b2, m2, v2, w3, g3, b3, m3, v3):
    f = np.float32
    bf = ml_dtypes.bfloat16
    f8 = ml_dtypes.float8_e4m3
    s1 = (g1 / np.sqrt(v1 + EPS)).astype(f)
    t1 = (b1 - m1 * s1).astype(f)
    s2 = (g2 / np.sqrt(v2 + EPS)).astype(f)
    t2 = (b2 - m2 * s2).astype(f)
    s3 = (g3 / np.sqrt(v3 + EPS)).astype(f)
    t3 = (b3 - m3 * s3).astype(f)

    # conv1 weights: fold s1, pre-scale x8, DR stationary layout
    # [ocb, p, kc2*256 + sub*128 + ocm]; cin = kc2*256 + sub*128 + p
    w1p = (w1 * s1[:, None] * WS).astype(f)               # [512, 2048]
    w18 = w1p.reshape(PC, 128, KC2, 2, 128).transpose(0, 4, 2, 3, 1)
    w18 = np.ascontiguousarray(w18.reshape(PC, 128, KC2 * 256)).astype(f8)

    # qk weights: [p, ocb(8), kc2*256 + sub*128 + ocm]
    wqk = (wqkv[:2 * P] * WS).astype(f)                   # [1024, 512]
    wqk8 = wqk.reshape(8, 128, 2, 2, 128).transpose(4, 0, 2, 3, 1)
    wqk8 = np.ascontiguousarray(wqk8.reshape(128, 8, 512)).astype(f8)

    wv = (wqkv[2 * P:] * WS).astype(f)                    # [512, 512]
    wvt = np.ascontiguousarray(
        wv.T.reshape(PC, 128, P).transpose(1, 0, 2)).astype(f8)
    w3p = (w3 * s3[:, None]).astype(f)                    # [2048, 512]
    w3t = np.ascontiguousarray(
        w3p.T.reshape(PC, 128, CIN).transpose(1, 0, 2)).astype(f8)
    pos = (rel_h + rel_w).reshape(P, N).astype(f)
    pos = np.ascontiguousarray(
        pos.reshape(PC, 128, N).transpose(1, 0, 2)).astype(bf)

    tb = np.concatenate([t1.reshape(PC, 128).T,
                         (s2 / WS).reshape(PC, 128).T], axis=1)
    tb = np.ascontiguousarray(tb, f)
    t2v = np.ascontiguousarray((WS * t2 / s2).reshape(1, P)).astype(bf)

    shared = dict(w18=w18, wqk8=wqk8, wvt=wvt, w3t=w3t, pos=pos, tb=tb,
                  t2v=t2v)

    xf = np.asarray(x, f)
    in_maps = []
    for c in range(NCORES):
        xc = xf[c * BPC:(c + 1) * BPC].reshape(BPC, CIN, N)
        # fp8 copy, partition-major DR layout:
        # [pair, p, kc2*784 + sub*392 + j*196 + n]
        x8 = xc.reshape(NPAIR, 2, KC2, 2, 128, N).transpose(0, 4, 2, 3, 1, 5)
        x8 = np.ascontiguousarray(
            x8.reshape(NPAIR, 128, KC2 * 784)).astype(f8)
        # bf16 residual copy with t3: [pair, p, kc1*392 + j*196 + n]
        xr = xc + t3[None, :, None]
        xr = xr.reshape(NPAIR, 2, KC1, 128, N).transpose(0, 3, 2, 1, 4)
        xr = np.ascontiguousarray(
            xr.reshape(NPAIR, 128, KC1 * N2)).astype(bf)
        in_maps.append(dict(shared, x8=x8, xr=xr))
    return in_maps


def _run(in_maps, trace=False, tmpdir=None):
    from concourse.bass_utils import run_bass_kernel_spmd
    if "nc" not in _CACHE:
        _CACHE["nc"] = _build()
    nc = _CACHE["nc"]
    return run_bass_kernel_spmd(nc, in_maps, core_ids=list(range(NCORES)),
                                trace=trace, tmpdir=tmpdir)


def _post(res):
    out = np.empty((B, CIN, H, W), np.float32)
    for c in range(NCORES):
        # y: [pair, p, kc1, j*196 + n]
        yc = res.results[c]["y"].astype(np.float32).reshape(
            NPAIR, 128, KC1, 2, N)
        out[c * BPC:(c + 1) * BPC] = yc.transpose(0, 3, 2, 1, 4).reshape(
            BPC, CIN, H, W)
    return out


def kernel(**inputs):
    in_maps = _prep_inputs(**inputs)
    res = _run(in_maps)
    return _post(res)
